# revision 14
# baseline (speedup 1.0000x reference)
"""Multi-head causal attention (B=2, C=2048, E=1024, H=16) on 8 NeuronCores.

Sharding: tensor-parallel over (batch, head-group): core = b*4 + g handles
batch b and heads [4g, 4g+4). Each core computes Q^T/K^T/V projections for
its 4 heads, causal attention, and its partial output projection
ctx_slice @ Wo_slice -> [2048, 1024]. Host sums the 4 partials per batch
(the tensor-parallel all-reduce, done at unshard time) and adds bo.

Dataflow is fully transposed so no on-device transposes are needed:
  Q^T = Wq_s.T @ x^T          [256 f, 2048 t]   (f = head-local features)
  K^T = Wk_s.T @ x^T          [256 f, 2048 t]
  V   = x    @ Wv_s           [2048 t, 256 f]  (natural layout, + ones col)
  s^T = K^T_h.T @ Q^T_h       [128 k, 512 q] per (head, k-chunk, q-tile)
  P^T = exp(s^T / 32) * mask  (no max-subtraction: |scores/32| < ~2.5)
  ctx_aug^T = V_aug.T @ P^T   [65, 512], row 64 = softmax normalizer l
  ctx^T = ctx_aug^T[0:64] * (1/l)  (broadcast via gpsimd)
  out_partial = ctx^T.T @ Wo_s     [2048, 1024] fp32

All matmul operands are fp16 (PE upconverts to FP22, accumulates fp32 in
PSUM): end-to-end max rel err vs fp64 reference is ~4e-4.
"""
import numpy as np

import concourse.bass as bass
import concourse.tile as tile
from concourse import bacc, mybir
from concourse.bass_utils import run_bass_kernel_spmd

F16 = mybir.dt.float16
F32 = mybir.dt.float32

B, C, E, H = 2, 2048, 1024, 16
NH = 4              # heads per core
D = 64              # head dim
FS = NH * D         # 256 features per core
EC = E // 128       # 8 e-chunks
QT = 512            # q tile size
NQ = C // QT        # 4 q tiles
KC = C // 128       # 16 k chunks
TC = C // 128       # 16 token chunks
SCALE = 1.0 / np.sqrt(np.float32(E))  # note: module scales by sqrt(E)

_CACHED_NC = None


def build():
    nc = bacc.Bacc("TRN2", target_bir_lowering=False, debug=False, num_devices=8)
    xT = nc.dram_tensor("xT", [E, C], F16, kind="ExternalInput")
    wq = nc.dram_tensor("wq", [E, FS], F16, kind="ExternalInput")
    wk = nc.dram_tensor("wk", [E, FS], F16, kind="ExternalInput")
    wv = nc.dram_tensor("wv", [E, FS], F16, kind="ExternalInput")
    wo = nc.dram_tensor("wo", [FS, E], F16, kind="ExternalInput")
    msk = nc.dram_tensor("msk", [4, 128, QT], F16, kind="ExternalInput")
    out = nc.dram_tensor("out", [C, E], F32, kind="ExternalOutput")

    with tile.TileContext(nc) as tc:
        with tc.tile_pool(name="const", bufs=1) as cp, \
             tc.tile_pool(name="work", bufs=1) as wp, \
             tc.tile_pool(name="ps", bufs=1, space="PSUM") as ps:
            # ---- resident SBUF tensors ----
            xT_sb = cp.tile([128, EC, C], F16)
            wq_sb = cp.tile([128, EC, FS], F16)
            wk_sb = cp.tile([128, EC, FS], F16)
            wv_sb = cp.tile([128, EC, FS], F16)
            wo_sb = cp.tile([128, 2, E], F16)
            msk_sb = cp.tile([128, 4, QT], F16)
            qt_sb = cp.tile([128, 2, C], F16)
            kt_sb = cp.tile([128, 2, C], F16)
            v_sb = cp.tile([128, TC, NH * (D + 1)], F16)   # +1: ones col per head
            ctxt_sb = cp.tile([128, 2, C], F16)

            # ---- input DMAs. DMA *issue* is serialized per issuing engine
            # (~640ns each), so spread across SP / ACT / GPSIMD queues and
            # order by first consumption (K-proj g2=0 j=0 needs wk + xT[0]).
            nc.scalar.dma_start(wk_sb[:], wk.rearrange("(c p) f -> p c f", p=128))
            for c in range(EC):
                nc.sync.dma_start(xT_sb[:, c, :], xT[c * 128:(c + 1) * 128, :])
            nc.scalar.dma_start(wq_sb[:], wq.rearrange("(c p) f -> p c f", p=128))
            nc.gpsimd.dma_start(wv_sb[:], wv.rearrange("(c p) f -> p c f", p=128))
            nc.gpsimd.dma_start(msk_sb[:], msk.rearrange("r p q -> p r q"))
            nc.gpsimd.dma_start(wo_sb[:], wo.rearrange("(g p) e -> p g e", p=128))
            nc.vector.memset(v_sb[:], 1.0)  # ones cols survive the V copy

            # ---- projections: K^T, Q^T ----
            for w_sb, o_sb in ((wk_sb, kt_sb), (wq_sb, qt_sb)):
                for g2 in range(2):
                    for j in range(NQ):
                        pp = ps.tile([128, QT], F32, tag="big", bufs=3,
                                     name=f"pp_{o_sb.name}_{g2}_{j}")
                        for c in range(EC):
                            nc.tensor.matmul(
                                pp[:],
                                lhsT=w_sb[:, c, 128 * g2:128 * (g2 + 1)],
                                rhs=xT_sb[:, c, QT * j:QT * (j + 1)],
                                start=(c == 0), stop=(c == EC - 1),
                            )
                        nc.vector.tensor_copy(
                            o_sb[:, g2, QT * j:QT * (j + 1)], pp[:])

            # ---- projection: V (natural layout, strided into per-head cols) --
            for t in range(TC):
                pp = ps.tile([128, FS], F32, tag="big", bufs=3,
                             name=f"pp_v_{t}")
                for c in range(EC):
                    nc.tensor.matmul(
                        pp[:],
                        lhsT=xT_sb[:, c, 128 * t:128 * (t + 1)],
                        rhs=wv_sb[:, c, :],
                        start=(c == 0), stop=(c == EC - 1),
                    )
                nc.vector.tensor_copy(
                    v_sb[:, t, :].rearrange("p (h x) -> p h x", h=NH)[:, :, 0:D],
                    pp[:].rearrange("p (h d) -> p h d", h=NH),
                )

            # ---- attention: head pairs (0,1)/(2,3); both heads' score tiles
            # ---- share one [128, 2*QT] psum so exp+mask are single wide ops
            def emit_scores(heads, j, c):
                """s^T pair -> one exp -> (one mask) -> fp16 P^T [128, 2*QT]."""
                st = ps.tile([128, 2 * QT], F32, tag="big", bufs=3,
                             name=f"st_{heads[0]}_{j}_{c}")
                for i, h in enumerate(heads):
                    g2, po = h // 2, 64 * (h % 2)
                    nc.tensor.matmul(
                        st[:, QT * i:QT * (i + 1)],
                        lhsT=kt_sb[po:po + 64, g2, 128 * c:128 * (c + 1)],
                        rhs=qt_sb[po:po + 64, g2, QT * j:QT * (j + 1)],
                        start=True, stop=True,
                    )
                pt = wp.tile([128, 2 * QT], F16, tag="pt", bufs=6)
                nc.scalar.activation(
                    pt[:], st[:], mybir.ActivationFunctionType.Exp, scale=SCALE)
                if c >= 4 * j:  # diagonal-straddling k-chunk: apply causal mask
                    ptm = wp.tile([128, 2 * QT], F16, tag="ptm", bufs=5)
                    nc.vector.tensor_mul(ptm[:], pt[:], msk_sb[:, c - 4 * j, :]
                                         .unsqueeze(1)
                                         .broadcast_to([128, 2, QT]))
                    return ptm
                return pt

            def attention(pair, j):
                heads = (2 * pair, 2 * pair + 1)
                nk = 4 * (j + 1)   # causal: k chunks 0..nk-1
                ctx_ps = {h: ps.tile([128, QT], F32, tag="ctx", bufs=2,
                                     name=f"ctx_{pair}_{j}_{h}")
                          for h in heads}
                pts = {}
                depth = min(3, nk)
                for c in range(depth):          # software-pipeline prologue
                    pts[c] = emit_scores(heads, j, c)
                for c in range(nk):
                    if c + depth < nk:
                        pts[c + depth] = emit_scores(heads, j, c + depth)
                    pt = pts.pop(c)
                    for i, h in enumerate(heads):
                        nc.tensor.matmul(
                            ctx_ps[h][0:D + 1, :],
                            lhsT=v_sb[:, c, (D + 1) * h:(D + 1) * (h + 1)],
                            rhs=pt[:, QT * i:QT * (i + 1)],
                            start=(c == 0), stop=(c == nk - 1),
                        )
                # normalize: ctx^T[d, q] * (1/l[q]).  Stage both heads' psum
                # to SBUF first (frees the ctx psum slots fast so following
                # matmuls aren't gated on the slow recip chain).
                stgs = {}
                for h in heads:
                    stg = wp.tile([D + 1, QT], F32, tag="stg", bufs=4,
                                  name=f"stg_{pair}_{j}_{h}")
                    nc.vector.tensor_copy(stg[:], ctx_ps[h][0:D + 1, :])
                    stgs[h] = stg
                lr = wp.tile([1, 2 * QT], F32, tag="lr", bufs=4)
                for i, h in enumerate(heads):
                    nc.vector.tensor_copy(lr[:, QT * i:QT * (i + 1)],
                                          stgs[h][D:D + 1, :])
                bc = wp.tile([64, 2 * QT], F32, tag="bc", bufs=4)
                nc.gpsimd.partition_broadcast(bc[:], lr[:])
                rc = wp.tile([64, 2 * QT], F32, tag="rc", bufs=4)
                nc.vector.reciprocal_approx_fast(rc[:], bc[:])
                for i, h in enumerate(heads):
                    g2, po = h // 2, 64 * (h % 2)
                    nc.vector.tensor_mul(
                        ctxt_sb[po:po + 64, g2, QT * j:QT * (j + 1)],
                        stgs[h][0:D, :], rc[:, QT * i:QT * (i + 1)])

            def wo_block(t):
                # partial out = ctx^T.T @ Wo_s for token chunk t
                for n in range(2):
                    pp = ps.tile([128, QT], F32, tag="ctx", bufs=2,
                                 name=f"pp_wo_{t}_{n}")
                    for g2 in range(2):
                        nc.tensor.matmul(
                            pp[:],
                            lhsT=ctxt_sb[:, g2, 128 * t:128 * (t + 1)],
                            rhs=wo_sb[:, g2, QT * n:QT * (n + 1)],
                            start=(g2 == 0), stop=(g2 == 1),
                        )
                    ot = wp.tile([128, QT], F32, tag="ot", bufs=4)
                    nc.vector.tensor_copy(ot[:], pp[:])
                    nc.sync.dma_start(
                        out[128 * t:128 * (t + 1), QT * n:QT * (n + 1)], ot[:])

            # interleave: both pairs for q-tile j, then that q-tile's Wo —
            # Wo matmuls fill PE stalls and the kernel tail shrinks to j=3's
            for j in range(NQ):
                attention(0, j)
                attention(1, j)
                for t in range(4 * j, 4 * (j + 1)):
                    wo_block(t)
    nc.compile()
    return nc


def _causal_masks():
    k = np.arange(128)[:, None]
    q = np.arange(QT)[None, :]
    return np.stack([(k + 128 * r <= q) for r in range(4)]).astype(np.float16)


def kernel(x, Wq, Wk, Wv, Wo, bo):
    global _CACHED_NC
    x = np.asarray(x, np.float32)
    Wq = np.asarray(Wq, np.float32)
    Wk = np.asarray(Wk, np.float32)
    Wv = np.asarray(Wv, np.float32)
    Wo = np.asarray(Wo, np.float32)
    bo = np.asarray(bo, np.float32)

    if _CACHED_NC is None:
        _CACHED_NC = build()
    nc = _CACHED_NC

    msk = _causal_masks()
    in_maps = []
    for b in range(B):
        xT_h = np.ascontiguousarray(x[b].T).astype(np.float16)
        for g in range(4):
            s = slice(g * FS, (g + 1) * FS)
            in_maps.append({
                "xT": xT_h,
                "wq": Wq[:, s].astype(np.float16),
                "wk": Wk[:, s].astype(np.float16),
                "wv": Wv[:, s].astype(np.float16),
                "wo": np.ascontiguousarray(Wo[s, :]).astype(np.float16),
                "msk": msk,
            })

    res = run_bass_kernel_spmd(nc, in_maps, core_ids=list(range(8)))

    out = np.empty((B, C, E), np.float32)
    for b in range(B):
        acc = res.results[b * 4 + 0]["out"].copy()
        for g in range(1, 4):
            acc += res.results[b * 4 + g]["out"]
        out[b] = acc + bo
    return out


# revision 15
# speedup vs baseline: 1.0591x; 1.0591x over previous
"""Multi-head causal attention (B=2, C=2048, E=1024, H=16) on 8 NeuronCores.

Sharding: tensor-parallel over (batch, head-group): core = b*4 + g handles
batch b and heads [4g, 4g+4). Each core computes Q^T/K^T/V projections for
its 4 heads, causal attention, and its partial output projection
ctx_slice @ Wo_slice -> [2048, 1024]. Host sums the 4 partials per batch
(the tensor-parallel all-reduce, done at unshard time) and adds bo.

Dataflow is fully transposed so no on-device transposes are needed:
  Q^T = Wq_s.T @ x^T          [256 f, 2048 t]   (f = head-local features)
  K^T = Wk_s.T @ x^T          [256 f, 2048 t]
  V   = x    @ Wv_s           [2048 t, 256 f]  (natural layout, + ones col)
  s^T = K^T_h.T @ Q^T_h       [128 k, 512 q] per (head, k-chunk, q-tile)
  P^T = exp(s^T / 32) * mask  (no max-subtraction: |scores/32| < ~2.5)
  ctx_aug^T = V_aug.T @ P^T   [65, 512], row 64 = softmax normalizer l
  ctx^T = ctx_aug^T[0:64] * (1/l)  (broadcast via gpsimd)
  out_partial = ctx^T.T @ Wo_s     [2048, 1024] fp32

All matmul operands are fp16 (PE upconverts to FP22, accumulates fp32 in
PSUM): end-to-end max rel err vs fp64 reference is ~4e-4.
"""
import numpy as np

import concourse.bass as bass
import concourse.tile as tile
from concourse import bacc, mybir
from concourse.bass_utils import run_bass_kernel_spmd

F16 = mybir.dt.float16
F32 = mybir.dt.float32

B, C, E, H = 2, 2048, 1024, 16
NH = 4              # heads per core
D = 64              # head dim
FS = NH * D         # 256 features per core
EC = E // 128       # 8 e-chunks
QT = 512            # q tile size
NQ = C // QT        # 4 q tiles
KC = C // 128       # 16 k chunks
TC = C // 128       # 16 token chunks
SCALE = 1.0 / np.sqrt(np.float32(E))  # note: module scales by sqrt(E)

_CACHED_NC = None


def build():
    nc = bacc.Bacc("TRN2", target_bir_lowering=False, debug=False, num_devices=8)
    xT = nc.dram_tensor("xT", [E, C], F16, kind="ExternalInput")
    wq = nc.dram_tensor("wq", [E, FS], F16, kind="ExternalInput")
    wk = nc.dram_tensor("wk", [E, FS], F16, kind="ExternalInput")
    wv = nc.dram_tensor("wv", [E, FS], F16, kind="ExternalInput")
    wo = nc.dram_tensor("wo", [FS, E], F16, kind="ExternalInput")
    msk = nc.dram_tensor("msk", [4, 128, QT], F16, kind="ExternalInput")
    out = nc.dram_tensor("out", [C, E], F32, kind="ExternalOutput")

    with tile.TileContext(nc) as tc:
        with tc.tile_pool(name="const", bufs=1) as cp, \
             tc.tile_pool(name="work", bufs=1) as wp, \
             tc.tile_pool(name="ps", bufs=1, space="PSUM") as ps:
            # ---- resident SBUF tensors ----
            xT_sb = cp.tile([128, EC, C], F16)
            wq_sb = cp.tile([128, EC, FS], F16)
            wk_sb = cp.tile([128, EC, FS], F16)
            wv_sb = cp.tile([128, EC, FS], F16)
            wo_sb = cp.tile([128, 2, E], F16)
            msk_sb = cp.tile([128, 4, QT], F16)
            qt_sb = cp.tile([128, 2, C], F16)
            kt_sb = cp.tile([128, 2, C], F16)
            v_sb = cp.tile([128, TC, NH * (D + 1)], F16)   # +1: ones col per head
            ctxt_sb = cp.tile([128, 2, C], F16)

            # ---- input DMAs. DMA *issue* is serialized per issuing engine
            # (~640ns each), so spread across SP / ACT / GPSIMD queues and
            # order by first consumption (K-proj g2=0 j=0 needs wk + xT[0]).
            nc.scalar.dma_start(wk_sb[:], wk.rearrange("(c p) f -> p c f", p=128))
            for c in range(EC):
                nc.sync.dma_start(xT_sb[:, c, :], xT[c * 128:(c + 1) * 128, :])
            nc.scalar.dma_start(wq_sb[:], wq.rearrange("(c p) f -> p c f", p=128))
            nc.gpsimd.dma_start(wv_sb[:], wv.rearrange("(c p) f -> p c f", p=128))
            nc.gpsimd.dma_start(msk_sb[:], msk.rearrange("r p q -> p r q"))
            nc.gpsimd.dma_start(wo_sb[:], wo.rearrange("(g p) e -> p g e", p=128))
            nc.vector.memset(v_sb[:], 1.0)  # ones cols survive the V copy

            # ---- projections: K^T, Q^T ----
            for w_sb, o_sb in ((wk_sb, kt_sb), (wq_sb, qt_sb)):
                for g2 in range(2):
                    for j in range(NQ):
                        pp = ps.tile([128, QT], F32, tag="big", bufs=3,
                                     name=f"pp_{o_sb.name}_{g2}_{j}")
                        for c in range(EC):
                            nc.tensor.matmul(
                                pp[:],
                                lhsT=w_sb[:, c, 128 * g2:128 * (g2 + 1)],
                                rhs=xT_sb[:, c, QT * j:QT * (j + 1)],
                                start=(c == 0), stop=(c == EC - 1),
                            )
                        nc.vector.tensor_copy(
                            o_sb[:, g2, QT * j:QT * (j + 1)], pp[:])

            # ---- projection: V (natural layout, strided into per-head cols) --
            for t in range(TC):
                pp = ps.tile([128, FS], F32, tag="big", bufs=3,
                             name=f"pp_v_{t}")
                for c in range(EC):
                    nc.tensor.matmul(
                        pp[:],
                        lhsT=xT_sb[:, c, 128 * t:128 * (t + 1)],
                        rhs=wv_sb[:, c, :],
                        start=(c == 0), stop=(c == EC - 1),
                    )
                nc.vector.tensor_copy(
                    v_sb[:, t, :].rearrange("p (h x) -> p h x", h=NH)[:, :, 0:D],
                    pp[:].rearrange("p (h d) -> p h d", h=NH),
                )

            # ---- attention: head pairs (0,1)/(2,3); both heads' score tiles
            # ---- share one [128, 2*QT] psum so exp+mask are single wide ops
            def emit_scores(heads, j, c):
                """s^T pair -> one exp -> (one mask) -> fp16 P^T [128, 2*QT]."""
                st = ps.tile([128, 2 * QT], F32, tag="big", bufs=3,
                             name=f"st_{heads[0]}_{j}_{c}")
                for i, h in enumerate(heads):
                    g2, po = h // 2, 64 * (h % 2)
                    nc.tensor.matmul(
                        st[:, QT * i:QT * (i + 1)],
                        lhsT=kt_sb[po:po + 64, g2, 128 * c:128 * (c + 1)],
                        rhs=qt_sb[po:po + 64, g2, QT * j:QT * (j + 1)],
                        start=True, stop=True,
                    )
                pt = wp.tile([128, 2 * QT], F16, tag="pt", bufs=6)
                nc.scalar.activation(
                    pt[:], st[:], mybir.ActivationFunctionType.Exp, scale=SCALE)
                if c >= 4 * j:  # diagonal-straddling k-chunk: apply causal mask
                    ptm = wp.tile([128, 2 * QT], F16, tag="ptm", bufs=5)
                    nc.vector.tensor_mul(ptm[:], pt[:], msk_sb[:, c - 4 * j, :]
                                         .unsqueeze(1)
                                         .broadcast_to([128, 2, QT]))
                    return ptm
                return pt

            def attention(pair, j):
                heads = (2 * pair, 2 * pair + 1)
                nk = 4 * (j + 1)   # causal: k chunks 0..nk-1
                ctx_ps = {h: ps.tile([128, QT], F32, tag="ctx", bufs=2,
                                     name=f"ctx_{pair}_{j}_{h}")
                          for h in heads}
                pts = {}
                depth = min(3, nk)
                for c in range(depth):          # software-pipeline prologue
                    pts[c] = emit_scores(heads, j, c)
                for c in range(nk):
                    if c + depth < nk:
                        pts[c + depth] = emit_scores(heads, j, c + depth)
                    pt = pts.pop(c)
                    for i, h in enumerate(heads):
                        nc.tensor.matmul(
                            ctx_ps[h][0:D + 1, :],
                            lhsT=v_sb[:, c, (D + 1) * h:(D + 1) * (h + 1)],
                            rhs=pt[:, QT * i:QT * (i + 1)],
                            start=(c == 0), stop=(c == nk - 1),
                        )
                # normalize: ctx^T[d, q] * (1/l[q]).  Stage both heads' psum
                # to SBUF first (frees the ctx psum slots fast so following
                # matmuls aren't gated on the slow recip chain).
                stgs = {}
                for h in heads:
                    stg = wp.tile([D + 1, QT], F32, tag="stg", bufs=4,
                                  name=f"stg_{pair}_{j}_{h}")
                    nc.vector.tensor_copy(stg[:], ctx_ps[h][0:D + 1, :])
                    stgs[h] = stg
                lr = wp.tile([1, 2 * QT], F32, tag="lr", bufs=4)
                for i, h in enumerate(heads):
                    nc.vector.tensor_copy(lr[:, QT * i:QT * (i + 1)],
                                          stgs[h][D:D + 1, :])
                bc = wp.tile([64, 2 * QT], F32, tag="bc", bufs=4)
                nc.gpsimd.partition_broadcast(bc[:], lr[:])
                rc = wp.tile([64, 2 * QT], F32, tag="rc", bufs=4)
                nc.vector.reciprocal_approx_fast(rc[:], bc[:])
                for i, h in enumerate(heads):
                    g2, po = h // 2, 64 * (h % 2)
                    nc.vector.tensor_mul(
                        ctxt_sb[po:po + 64, g2, QT * j:QT * (j + 1)],
                        stgs[h][0:D, :], rc[:, QT * i:QT * (i + 1)])

            def wo_block(t):
                # partial out = ctx^T.T @ Wo_s for token chunk t
                for n in range(2):
                    pp = ps.tile([128, QT], F32, tag="ctx", bufs=2,
                                 name=f"pp_wo_{t}_{n}")
                    for g2 in range(2):
                        nc.tensor.matmul(
                            pp[:],
                            lhsT=ctxt_sb[:, g2, 128 * t:128 * (t + 1)],
                            rhs=wo_sb[:, g2, QT * n:QT * (n + 1)],
                            start=(g2 == 0), stop=(g2 == 1),
                        )
                    ot = wp.tile([128, QT], F32, tag="ot", bufs=4)
                    nc.vector.tensor_copy(ot[:], pp[:])
                    nc.sync.dma_start(
                        out[128 * t:128 * (t + 1), QT * n:QT * (n + 1)], ot[:])

            # interleave: both pairs for q-tile j, then the PREVIOUS q-tile's
            # Wo (delayed one j so its normalize chain has long finished) —
            # Wo matmuls fill PE stalls and the kernel tail shrinks to j=3's
            for j in range(NQ):
                attention(0, j)
                attention(1, j)
                if j > 0:
                    for t in range(4 * (j - 1), 4 * j):
                        wo_block(t)
            for t in range(4 * (NQ - 1), 4 * NQ):
                wo_block(t)
    nc.compile()
    return nc


def _causal_masks():
    k = np.arange(128)[:, None]
    q = np.arange(QT)[None, :]
    return np.stack([(k + 128 * r <= q) for r in range(4)]).astype(np.float16)


def kernel(x, Wq, Wk, Wv, Wo, bo):
    global _CACHED_NC
    x = np.asarray(x, np.float32)
    Wq = np.asarray(Wq, np.float32)
    Wk = np.asarray(Wk, np.float32)
    Wv = np.asarray(Wv, np.float32)
    Wo = np.asarray(Wo, np.float32)
    bo = np.asarray(bo, np.float32)

    if _CACHED_NC is None:
        _CACHED_NC = build()
    nc = _CACHED_NC

    msk = _causal_masks()
    in_maps = []
    for b in range(B):
        xT_h = np.ascontiguousarray(x[b].T).astype(np.float16)
        for g in range(4):
            s = slice(g * FS, (g + 1) * FS)
            in_maps.append({
                "xT": xT_h,
                "wq": Wq[:, s].astype(np.float16),
                "wk": Wk[:, s].astype(np.float16),
                "wv": Wv[:, s].astype(np.float16),
                "wo": np.ascontiguousarray(Wo[s, :]).astype(np.float16),
                "msk": msk,
            })

    res = run_bass_kernel_spmd(nc, in_maps, core_ids=list(range(8)))

    out = np.empty((B, C, E), np.float32)
    for b in range(B):
        acc = res.results[b * 4 + 0]["out"].copy()
        for g in range(1, 4):
            acc += res.results[b * 4 + g]["out"]
        out[b] = acc + bo
    return out
